# revision 6
# baseline (speedup 1.0000x reference)
"""Trainium2 Bass kernel for nn_CausalSelfAttention_8237747274097 — v2.

All-fp8 DoubleRow rewrite with residual compensation.

Math (exact folds, as v1):
    qkv = x @ W_attn + b_attn ; q,k,v = split ; single-head attention.
    Wqk = (Wq/8) @ Wk^T folded (no K projection);  Wvp = Wv @ W_proj folded
    (no output projection); k-bias drops, v-bias folds into host b_eff.
    Per-key score bias t_j = (bq/8)·Wk^T·x_j enters via a K=4 init matmul.

Numerics: every matmul runs as fp8e4m3 DoubleRow (2 k-chunks per
instruction, 0.5 cycles/out-elem) with hi+lo residual compensation:
for operands A≈Ah+Al, B≈Bh+Bl the product uses 3 chains Ah·Bh + Al·Bh +
Ah·Bl accumulated in one PSUM (residuals are UNSCALED e4m3 — fp8
subnormals verified exact on HW).  exp outputs are split as e5m2 hi+lo
(ph + pl = p exactly to ~1.6%).  Verified end-to-end metric 2.8e-3 vs
the 2e-2 gate in numpy emulation.

Scales (exact pow-2): Wqk*512 (scores descaled inside exp), Wvp*64
(descaled via the den=64*sum(p) reciprocal).  Global exp shift -2.75
cancels in softmax and keeps exp(s) inside e5m2 range.

Causal masking: dead 128-col regions are killed by the t-init matmul
itself (row2 = -240 times a data-selected 240-pattern => psS <= -5e4 =>
exp == 0 exactly); diagonal tiles get a triangular multiply on the f32
exp output.  All parity differences are DATA (mov_sel / dmask), so one
NEFF serves all 8 cores.

Sharding (unchanged from v1): core c = (batch c//2, parity c%2); each
core owns 8 of 16 query row-tiles (OWN), computes full V for its batch.
"""

import numpy as np
import ml_dtypes

import concourse.bass as bass
import concourse.tile as tile
import concourse.mybir as mybir
from concourse import bacc
from concourse.bass import ts, ds
from concourse.bass_utils import run_bass_kernel_spmd

F32 = mybir.dt.float32
F16 = mybir.dt.float16
E4 = mybir.dt.float8e4
E5 = mybir.dt.float8e5
DR = mybir.MatmulPerfMode.DoubleRow
NE4 = ml_dtypes.float8_e4m3
NE5 = ml_dtypes.float8_e5m2

T, D = 2048, 1024
NT = T // 128          # 16 key/query tiles
DP = 4                 # d-chunk pairs (8 chunks of 128, DoubleRow-paired)
OWN = [[15, 12, 11, 8, 7, 4, 3, 0],
       [14, 13, 10, 9, 6, 5, 2, 1]]
CP = [16, 12, 8, 4]    # j-blocks per pair P (uniform across cores)
PAIR_ORDER = (0, 1, 2, 3)
SQ = 512.0
SV = 64.0
CSHIFT = 2.75          # exact in fp8/f32; exp(s - CSHIFT)
NBLK = sum(CP) // 2    # 20 tj-pair blocks per core

_NC_CACHE = {}


def _build(repeat=1, phases=3):
    key = (repeat, phases)
    if key in _NC_CACHE:
        return _NC_CACHE[key]
    nc = bacc.Bacc("TRN2", target_bir_lowering=False, debug=False,
                   enable_asserts=False, num_devices=8)
    t = {}
    for nm in ("xth", "xtl"):
        t[nm] = nc.dram_tensor(nm, [DP, 128, 2, T], E4, kind="ExternalInput").ap()
    for nm in ("xqh", "xql", "wqh", "wql", "wvh", "wvl"):
        t[nm] = nc.dram_tensor(nm, [DP, 128, 2, 1024], E4, kind="ExternalInput").ap()
    t["tst"] = nc.dram_tensor("tst", [2, 2, 1024], E4, kind="ExternalInput").ap()
    t["mov_sel"] = nc.dram_tensor("mov_sel", [2, NBLK, 2, 512], E4,
                                  kind="ExternalInput").ap()
    t["dmask"] = nc.dram_tensor("dmask", [2, 2, 128, 128], F32,
                                kind="ExternalInput").ap()
    t["out"] = nc.dram_tensor("out", [1024, 1024], F16, kind="ExternalOutput").ap()

    with tile.TileContext(nc, pool_alloc_mode="queue") as tc:
        def body(_i=None):
            _emit(nc, tc, t, phases)
        if repeat == 1:
            body()
        else:
            with tc.For_i(0, repeat, 1):
                body()
    nc.compile()
    _NC_CACHE[key] = nc
    return nc


def _emit(nc, tc, t, phases=3):
    with tc.tile_pool(name="xt", bufs=1) as xt_pool, \
         tc.tile_pool(name="xq", bufs=1) as xq_pool, \
         tc.tile_pool(name="wq", bufs=1) as wq_pool, \
         tc.tile_pool(name="wv", bufs=1) as wv_pool, \
         tc.tile_pool(name="gp", bufs=1) as g_pool, \
         tc.tile_pool(name="vp", bufs=1) as v_pool, \
         tc.tile_pool(name="small", bufs=1) as small:

        xth = [xt_pool.tile([128, 2, T], E4, tag=f"xth{d}", name=f"xth{d}")
               for d in range(DP)]
        xtl = [xt_pool.tile([128, 2, T], E4, tag=f"xtl{d}", name=f"xtl{d}")
               for d in range(DP)]
        xqh = [xq_pool.tile([128, 2, 1024], E4, tag=f"xqh{d}", name=f"xqh{d}")
               for d in range(DP)]
        xql = [xq_pool.tile([128, 2, 1024], E4, tag=f"xql{d}", name=f"xql{d}")
               for d in range(DP)]
        wqh = [wq_pool.tile([128, 2, 1024], E4, tag=f"wqh{d}", name=f"wqh{d}")
               for d in range(DP)]
        wql = [wq_pool.tile([128, 2, 1024], E4, tag=f"wql{d}", name=f"wql{d}")
               for d in range(DP)]
        wvh = [wv_pool.tile([128, 2, 1024], E4, tag=f"wvh{d}", name=f"wvh{d}")
               for d in range(DP)]
        wvl = [wv_pool.tile([128, 2, 1024], E4, tag=f"wvl{d}", name=f"wvl{d}")
               for d in range(DP)]
        qph = [g_pool.tile([128, 2, 1024], E4, tag=f"qph{m}", name=f"qph{m}")
               for m in range(DP)]
        qpl = [g_pool.tile([128, 2, 1024], E4, tag=f"qpl{m}", name=f"qpl{m}")
               for m in range(DP)]
        vph = [v_pool.tile([128, 2, 1024], E4, tag=f"vph{p}", name=f"vph{p}")
               for p in range(NT // 2)]
        vpl = [v_pool.tile([128, 2, 1024], E4, tag=f"vpl{p}", name=f"vpl{p}")
               for p in range(NT // 2)]
        tst = small.tile([2, 2, 1024], E4, tag="tst", name="tst_sb")
        mov = small.tile([2, NBLK, 2, 512], E4, tag="mov", name="mov_sb")
        dmsk = small.tile([128, 2, 2, 128], F32, tag="dmsk", name="dmsk_sb")
        ones64 = small.tile([128, 2, 1], E4, tag="ones", name="ones64")
        cbias = small.tile([128, 1], F32, tag="cb", name="cbias")
        nc.vector.memset(ones64[:], SV)
        nc.vector.memset(cbias[:], -CSHIFT)
        wrm = small.tile([128, 1], F32, tag="wrm", name="wrm")
        nc.scalar.activation(wrm[:], cbias[:],
                             mybir.ActivationFunctionType.Exp)

        # ---- DMA choreography (SP queue is in-order) ----
        # Q cold start first, then the rest in consumption order.
        nc.sync.dma_start(wqh[0][:, :, 0:384], t["wqh"][0, :, :, 0:384])
        # cold start: x slivers ride the Pool/SWDGE path, bypassing the
        # shared HWDGE issue device so their latency overlaps DMA#1's
        nc.gpsimd.dma_start(xqh[0][:, :, 0:512], t["xqh"][0, :, :, 0:512])
        nc.gpsimd.dma_start(xqh[0][:, :, 512:1024], t["xqh"][0, :, :, 512:1024])
        nc.sync.dma_start(wqh[0][:, :, 384:1024], t["wqh"][0, :, :, 384:1024])
        nc.sync.dma_start(wql[0][:], t["wql"][0, :, :, :])
        nc.sync.dma_start(xql[0][:], t["xql"][0, :, :, :])
        for d in range(1, DP):
            nc.sync.dma_start(wqh[d][:], t["wqh"][d, :, :, :])
            nc.sync.dma_start(xqh[d][:], t["xqh"][d, :, :, :])
            nc.sync.dma_start(wql[d][:], t["wql"][d, :, :, :])
            nc.sync.dma_start(xql[d][:], t["xql"][d, :, :, :])
        for d in range(DP):
            nc.sync.dma_start(wvh[d][:], t["wvh"][d, :, :, :])
        for d in range(DP):
            nc.sync.dma_start(xth[d][:, :, 0:1024], t["xth"][d, :, :, 0:1024])
        for d in range(DP):
            nc.sync.dma_start(wvl[d][:], t["wvl"][d, :, :, :])
        for d in range(DP):
            nc.sync.dma_start(xtl[d][:, :, 0:1024], t["xtl"][d, :, :, 0:1024])
        for d in range(DP):
            nc.sync.dma_start(xth[d][:, :, 1024:2048], t["xth"][d, :, :, 1024:2048])
            nc.sync.dma_start(xtl[d][:, :, 1024:2048], t["xtl"][d, :, :, 1024:2048])
        nc.sync.dma_start(tst[:], t["tst"][:, :, :])
        nc.sync.dma_start(mov[:], t["mov_sel"][:, :, :, :])
        # dmask dram [2,2,128,128] -> sbuf [128, 2, 2, 128]
        nc.sync.dma_start(dmsk[:], t["dmask"].rearrange("a b p c -> p a b c"))

        pt_cm = tc.tile_pool(name="ptp", bufs=1)
        trans_cm = tc.tile_pool(name="trans", bufs=1)
        psS_cm = tc.tile_pool(name="psS", bufs=1, space="PSUM")
        pt_pool = pt_cm.__enter__()
        trans = trans_cm.__enter__()
        psS_pool = psS_cm.__enter__()
        psA_cm = tc.tile_pool(name="psA", bufs=1, space="PSUM")
        psA = psA_cm.__enter__()

        # ---- Phase Q: G = x_q @ Wqk_s, stored as e4m3 hi+lo pairs ----
        CHAINS_Q = ((wqh, xqh), (wql, xqh), (wqh, xql))

        def q_copies(m, half, ps):
            mp, sub = m // 2, m % 2
            dst_h = qph[mp][:, sub, ds(512 * half, 512)]
            dst_l = qpl[mp][:, sub, ds(512 * half, 512)]
            nc.scalar.activation(dst_h, ps[:],
                                 mybir.ActivationFunctionType.Copy)
            nc.vector.tensor_sub(dst_l, ps[:], dst_h)

        # group0: dp-outer to stream arriving tiles; last dp sweep m-first
        grp = ((0, 0), (0, 1), (1, 0), (1, 1), (2, 0), (2, 1))
        pss = {mh: psA.tile([128, 512], F32, tag="A", name="psQ_t", bufs=6)
               for mh in grp}
        for dp in range(DP - 1):
            for ci, (lh, rh) in enumerate(CHAINS_Q):
                for (m, half) in grp:
                    nc.tensor.matmul(
                        pss[(m, half)][:],
                        lh[dp][:, :, ts(m, 128)],
                        rh[dp][:, :, ds(512 * half, 512)],
                        start=(dp == 0 and ci == 0),
                        stop=False,
                        perf_mode=DR)
        for (m, half) in grp:
            for ci, (lh, rh) in enumerate(CHAINS_Q):
                nc.tensor.matmul(
                    pss[(m, half)][:],
                    lh[DP - 1][:, :, ts(m, 128)],
                    rh[DP - 1][:, :, ds(512 * half, 512)],
                    start=False, stop=(ci == 2),
                    perf_mode=DR)
            q_copies(m, half, pss[(m, half)])
        # m 3..7: streamed half-tiles (all inputs resident by now)
        for m in range(3, 8):
            for half in range(2):
                ps = psA.tile([128, 512], F32, tag="A", name="psQ_t", bufs=6)
                for dp in range(DP):
                    for ci, (lh, rh) in enumerate(CHAINS_Q):
                        nc.tensor.matmul(
                            ps[:],
                            lh[dp][:, :, ts(m, 128)],
                            rh[dp][:, :, ds(512 * half, 512)],
                            start=(dp == 0 and ci == 0),
                            stop=(dp == DP - 1 and ci == 2),
                            perf_mode=DR)
                q_copies(m, half, ps)

        # ---- Phase V: VP = x @ Wvp_s (full batch), e4m3 hi+lo pairs ----
        CHAINS_V = ((xth, wvh), (xtl, wvh), (xth, wvl))
        for tt in range(NT):
            tp, sub = tt // 2, tt % 2
            for half in range(2):
                psV = psA.tile([128, 512], F32, tag="A", name="psV_t", bufs=6)
                for ci, (lh, rh) in enumerate(CHAINS_V):
                    for dp in range(DP):
                        nc.tensor.matmul(
                            psV[:],
                            lh[dp][:, :, ts(tt, 128)],
                            rh[dp][:, :, ds(512 * half, 512)],
                            start=(ci == 0 and dp == 0),
                            stop=(ci == 2 and dp == DP - 1),
                            perf_mode=DR)
                dst_h = vph[tp][:, sub, ds(512 * half, 512)]
                dst_l = vpl[tp][:, sub, ds(512 * half, 512)]
                nc.scalar.activation(dst_h, psV[:],
                                     mybir.ActivationFunctionType.Copy)
                nc.vector.tensor_sub(dst_l, psV[:], dst_h)

        if phases <= 1:
            psA_cm.__exit__(None, None, None)
            with tc.tile_pool(name="dump", bufs=1) as dump:
                tk = dump.tile([128, 512], F16, tag="tk", name="tk")
                nc.vector.tensor_copy(tk[:], vph[0][:, 0, 0:512])
                nc.sync.dma_start(t["out"][0:128, 0:512], tk[:])
            return

        # ---- Phase B: scores -> exp split -> PV + den, software-pipelined ----
        # The first pair's scores are emitted while the psA pool is still
        # open (psS takes the 2 spare PSUM banks), so phase B overlaps the V
        # tail; po/pden pools open only after psA closes.
        blk_base = {}
        acc = 0
        for P in PAIR_ORDER:
            blk_base[P] = acc
            acc += CP[P] // 2

        po_pool = pden_pool = None

        if True:
            CHAINS_S = ((xth, qph), (xtl, qph), (xth, qpl))

            def emit_scores(P):
                cp = CP[P]
                blocks = cp // 2
                pts = []
                for tjp in range(blocks):
                    bid = blk_base[P] + tjp
                    psS = psS_pool.tile([128, 2, 256], F32, tag="s",
                                        name="psS_t", bufs=2)
                    nc.tensor.matmul(psS[:, :, :], tst[:, :, ts(tjp, 128)],
                                     mov[:, bid, :, :], start=True, stop=False,
                                     perf_mode=DR)
                    wS = 128 if tjp == blocks - 1 else 256
                    for i in range(2):
                        tj = 2 * tjp + i
                        for dp in range(DP):
                            for ci, (lh, rh) in enumerate(CHAINS_S):
                                nc.tensor.matmul(
                                    psS[:, i, 0:wS],
                                    lh[dp][:, :, ts(tj, 128)],
                                    rh[dp][:, :, ds(P * 256, wS)],
                                    start=False,
                                    stop=(dp == DP - 1 and ci == 2),
                                    perf_mode=DR)
                    p32 = trans.tile([128, 2, 256], F32, tag="p32",
                                     name="p32_t", bufs=4)
                    nc.scalar.activation(p32[:, :, :], psS[:, :, :],
                                         mybir.ActivationFunctionType.Exp,
                                         bias=cbias[:, 0:1], scale=1.0 / SQ)
                    if tjp == blocks - 2:      # block A masks
                        nc.vector.tensor_mul(p32[:, 0, 128:256],
                                             p32[:, 0, 128:256],
                                             dmsk[:, 0, 0, :])
                        nc.vector.tensor_mul(p32[:, 1, 128:256],
                                             p32[:, 1, 128:256],
                                             dmsk[:, 0, 1, :])
                    if tjp == blocks - 1:      # block B masks
                        nc.vector.tensor_mul(p32[:, 0, 0:128],
                                             p32[:, 0, 0:128],
                                             dmsk[:, 1, 0, :])
                        nc.vector.tensor_mul(p32[:, 1, 0:128],
                                             p32[:, 1, 0:128],
                                             dmsk[:, 1, 1, :])
                    pth = pt_pool.tile([128, 2, 256], E5, tag=f"pth{tjp}",
                                       name="pth_t", bufs=2)
                    ptl = pt_pool.tile([128, 2, 256], E5, tag=f"ptl{tjp}",
                                       name="ptl_t", bufs=2)
                    nc.scalar.activation(pth[:, :, :], p32[:, :, :],
                                         mybir.ActivationFunctionType.Copy)
                    nc.vector.tensor_sub(ptl[:, :, :], p32[:, :, :],
                                         pth[:, :, :])
                    pts.append((pth, ptl))
                return pts

            def emit_pv(P, pts):
                if phases <= 2:
                    return
                cp = CP[P]
                blocks = cp // 2
                for slot in range(2):
                    nblk = blocks if slot == 0 else blocks - 1
                    row = 128 * (2 * P + slot)
                    pden = pden_pool.tile([128, 1], F32, tag=f"pd{slot}",
                                          name=f"pden{slot}_t", bufs=1)
                    rec = trans.tile([128, 1], F32, tag="rec", name="rec_t",
                                     bufs=4)
                    ob = trans.tile([128, 1024], F16, tag="ob",
                                    name="ob_t", bufs=3)
                    for half in range(2):
                        po = po_pool.tile([128, 512], F32, tag="po",
                                          name="po_t", bufs=2)
                        for bi in range(nblk):
                            ph, pl = pts[bi]
                            sh = ph[:, :, ds(slot * 128, 128)]
                            sl = pl[:, :, ds(slot * 128, 128)]
                            if half == 0:
                                nc.tensor.matmul(pden[:], sh, ones64[:, :, :],
                                                 start=(bi == 0), stop=False,
                                                 perf_mode=DR)
                                nc.tensor.matmul(pden[:], sl, ones64[:, :, :],
                                                 start=False,
                                                 stop=(bi == nblk - 1),
                                                 perf_mode=DR)
                            mh = vph[bi][:, :, ds(512 * half, 512)]
                            ml = vpl[bi][:, :, ds(512 * half, 512)]
                            for ci, (s_, m_) in enumerate(
                                    ((sh, mh), (sl, mh), (sh, ml))):
                                nc.tensor.matmul(
                                    po[:], s_, m_,
                                    start=(bi == 0 and ci == 0),
                                    stop=(bi == nblk - 1 and ci == 2),
                                    perf_mode=DR)
                        if half == 0:
                            nc.vector.reciprocal(rec[:], pden[:])
                            nc.scalar.activation(
                                ob[:, 0:512], po[:],
                                mybir.ActivationFunctionType.Copy,
                                bias=0.0, scale=rec[:, 0:1])
                        else:
                            nc.vector.tensor_scalar_mul(ob[:, 512:1024], po[:],
                                                        rec[:, 0:1])
                        if P == PAIR_ORDER[-1]:
                            # tail pair: per-half DMAs so half0 departs early
                            # and the final transfer on the critical path is
                            # half-size
                            nc.sync.dma_start(
                                t["out"][ds(row, 128), ds(512 * half, 512)],
                                ob[:, ds(512 * half, 512)])
                    if P != PAIR_ORDER[-1]:
                        nc.sync.dma_start(t["out"][ds(row, 128), :], ob[:])

            prev = None
            for P in PAIR_ORDER:
                pts = emit_scores(P)
                if po_pool is None:
                    # first pair's scores emitted; now V is long done --
                    # release psA and claim its banks for PV accumulators
                    psA_cm.__exit__(None, None, None)
                    po_cm = tc.tile_pool(name="poP", bufs=1, space="PSUM")
                    pden_cm = tc.tile_pool(name="pdn", bufs=1, space="PSUM")
                    po_pool = po_cm.__enter__()
                    pden_pool = pden_cm.__enter__()
                if prev is not None:
                    emit_pv(*prev)
                prev = (P, pts)
            emit_pv(*prev)
            pden_cm.__exit__(None, None, None)
            po_cm.__exit__(None, None, None)
        psS_cm.__exit__(None, None, None)
        trans_cm.__exit__(None, None, None)
        pt_cm.__exit__(None, None, None)


def _split8(a):
    """f32 array -> (hi, lo) e4m3 pair with unscaled residual."""
    hi = a.astype(NE4)
    lo = (a - hi.astype(np.float32)).astype(NE4)
    return hi, lo


def _wlayout(w):
    # [1024 (d), 1024 (m)] -> [DP, 128, 2, 1024]: pair d-chunks for DoubleRow
    return np.ascontiguousarray(
        w.reshape(DP, 2, 128, 1024).transpose(0, 2, 1, 3))


def _host_consts(par):
    """mov_sel (t-init moving patterns) and dmask for a parity."""
    v = np.zeros((4, 4, 512), np.float32)
    for k in range(4):
        v[k, 0, 0:256] = 1.0
        v[k, 1, 256:512] = 1.0
    v[1, 2, 384:512] = 240.0                      # parity0 block A
    v[2, 2, 128:256] = 240.0                      # parity0 block B
    v[2, 2, 384:512] = 240.0
    v[3, 2, 128:512] = 240.0                      # parity1 block B
    mov = np.zeros((NBLK, 4, 512), np.float32)
    bi = 0
    for P in PAIR_ORDER:
        blocks = CP[P] // 2
        for tjp in range(blocks):
            if par == 0 and tjp == blocks - 2:
                sel = 1
            elif par == 0 and tjp == blocks - 1:
                sel = 2
            elif par == 1 and tjp == blocks - 1:
                sel = 3
            else:
                sel = 0
            mov[bi] = v[sel]
            bi += 1
    assert bi == NBLK
    # DoubleRow layout [2(k), NBLK, 2(i), 512]: k0i0=row0, k1i0=row2(kill),
    # k0i1=row1, k1i1=0
    mov_dr = np.zeros((2, NBLK, 2, 512), np.float32)
    mov_dr[0, :, 0, :] = mov[:, 0, :]
    mov_dr[1, :, 0, :] = mov[:, 2, :]
    mov_dr[0, :, 1, :] = mov[:, 1, :]
    mov = mov_dr

    tri = (np.arange(128)[:, None] <= np.arange(128)[None, :]).astype(np.float32)
    ones = np.ones((128, 128), np.float32)
    dm = np.empty((2, 2, 128, 128), np.float32)
    if par == 0:
        dm[0, 0], dm[0, 1] = tri, ones            # block A: diag at [i=0,slot1]
        dm[1, 0], dm[1, 1] = ones, tri            # block B: diag at [i=1,slot0]
    else:
        dm[0, 0], dm[0, 1] = ones, tri            # block A: diag at [i=1,slot1]
        dm[1, 0], dm[1, 1] = tri, ones            # block B: diag at [i=0,slot0]
    return mov.astype(NE4), dm


def kernel(x, W_attn, b_attn, W_proj, b_proj, _repeat=1, _results_only=False,
           _phases=3):
    x = np.asarray(x, np.float32)
    W_attn = np.asarray(W_attn, np.float64)
    b_attn = np.asarray(b_attn, np.float64)
    W_proj = np.asarray(W_proj, np.float64)
    b_proj = np.asarray(b_proj, np.float64)
    B = x.shape[0]

    nc = _build(_repeat, _phases)

    Wq = W_attn[:, :D]
    Wk = W_attn[:, D:2 * D]
    Wv = W_attn[:, 2 * D:]
    wqk_s = np.float32((Wq * 0.125) @ Wk.T * SQ)
    wvp_s = np.float32(Wv @ W_proj * SV)
    bqk = (b_attn[:D] * 0.125) @ Wk.T            # [D], f64
    b_eff = np.float32(b_proj + b_attn[2 * D:] @ W_proj)

    wqh, wql = _split8(wqk_s)
    wvh, wvl = _split8(wvp_s)
    wqh, wql = _wlayout(wqh), _wlayout(wql)
    wvh, wvl = _wlayout(wvh), _wlayout(wvl)
    consts = [_host_consts(0), _host_consts(1)]

    def xlayout(a, n):
        # [1024 (d), n (tok)] e4 -> [DP, 128, 2, n]
        return np.ascontiguousarray(
            a.reshape(DP, 2, 128, n).transpose(0, 2, 1, 3))

    in_maps = []
    for c in range(8):
        b, par = c // 2, c % 2
        own = OWN[par]
        xTb = np.ascontiguousarray(x[b].T)       # [D, T] f32
        xh, xl = _split8(xTb)
        cols = np.concatenate([np.arange(128 * q, 128 * (q + 1)) for q in own])
        tvec = np.float32(x[b].astype(np.float64) @ bqk) * np.float32(SQ)
        trows = tvec.reshape(8, 2, 128).transpose(1, 0, 2).reshape(2, 1024)
        tst = np.zeros((2, 2, 1024), np.float32)
        tst[0, 0] = trows[0]        # k0,i0: t(tj0)
        tst[1, 0] = -240.0          # k1,i0: kill row
        tst[0, 1] = trows[1]        # k0,i1: t(tj1)
        mov, dm = consts[par]
        in_maps.append({
            "xth": xlayout(xh, T), "xtl": xlayout(xl, T),
            "xqh": xlayout(np.ascontiguousarray(xh[:, cols]), 1024),
            "xql": xlayout(np.ascontiguousarray(xl[:, cols]), 1024),
            "wqh": wqh, "wql": wql, "wvh": wvh, "wvl": wvl,
            "tst": tst.astype(NE4), "mov_sel": mov, "dmask": dm,
        })

    res = run_bass_kernel_spmd(nc, in_maps, core_ids=list(range(8)))
    if _results_only:
        return res

    out = np.empty((B, T, D), np.float32)
    for c in range(8):
        b, par = c // 2, c % 2
        part = res.results[c]["out"].astype(np.float32)
        for s, q in enumerate(OWN[par]):
            out[b, 128 * q:128 * (q + 1), :] = part[128 * s:128 * (s + 1), :] + b_eff
    return out
